# revision 15
# baseline (speedup 1.0000x reference)
"""GATv2Conv (heads=1, edge_dim=11, add_self_loops fill='mean') on 8 Trainium2 cores.

Sharding: nodes dealt to cores by degree rank (core = rank % 8), so per-cell
edge counts are nearly identical across cores and the shared-NEFF padding is
minimal. Edges grouped per (dst-chunk, src-bank) cell; per-edge xl[src] rows
come from one dma_gather per cell out of an on-device bf16 table C[n]=[xl|xr].
Self-loops are folded in as ordinary edges (attr = host-computed mean).

Per chunk, tiles are processed in groups of 8: one-hot S built by a single DVE
compare, transposed on the PE (batched PSUM->SBUF copy), messages assembled in
PSUM via bf16 matmuls only (block-diagonal attr@We + S^T@xr + identity@xl),
softmax weights applied, and segment sums done by S one-hot matmuls.

All fp32 has been removed from the DMA and matmul paths (bf16 end to end,
fp32 only in PSUM accumulation and the final divide). Host work is layout
only: sharding, grouping, index packing, casts, and the self-loop attr mean.
"""

import sys

sys.path.insert(0, "/opt/trn_rl_repo")

import numpy as np
import ml_dtypes

import concourse.bass as bass
import concourse.bacc as bacc
import concourse.tile as tile
import concourse.mybir as mybir
from concourse.bass_utils import run_bass_kernel_spmd

BF16 = ml_dtypes.bfloat16
AF = mybir.ActivationFunctionType
OP = mybir.AluOpType

N, E, DIN, DOUT, DE = 1000_00, 1000_000, 128, 64, 11
NEG_SLOPE = 0.2
NC = 8
NOWN = N // NC                    # 12500
NCHUNK = 98                       # ceil(12500/128)
NPAD = NCHUNK * 128               # 12544
BANKS = 4
BANKROWS = 25024                  # 4*25024 = 100096 >= N; < 2^15 for int16 idx
NSLAB = 49                        # table build slabs of 2048 rows
NTAB = NSLAB * 2048               # 100352 >= 100096
GS = 8                            # tiles per processing group
RHS_W = 65                        # [ex | ex*xl]

last_exec_time_ns = None
last_result = None
_CACHE = {}


def _cdiv(a, b):
    return -(-a // b)


def _bc3(ap2, mid):
    """[P, F] AP -> [P, mid, F] AP broadcast along a new middle dim."""
    return bass.AP(ap2.tensor, ap2.offset, [ap2.ap[0], [0, mid], ap2.ap[1]])


def _in3(ap2, inner):
    """[P, T] AP -> [P, T, inner] AP broadcast along a new inner dim."""
    return bass.AP(ap2.tensor, ap2.offset, [ap2.ap[0], ap2.ap[1], [0, inner]])


# --------------------------------------------------------------------------
# host-side layout (index manipulation only)
# --------------------------------------------------------------------------

def _plan(edge_index):
    src = np.asarray(edge_index[0]).astype(np.int64)
    dst = np.asarray(edge_index[1]).astype(np.int64)
    deg = np.bincount(dst, minlength=N).astype(np.int64)

    # deal nodes to cores by degree rank: nearly equal per-cell counts
    order = np.argsort(deg, kind="stable")        # node ids, ascending degree
    core_of = np.empty(N, np.int64)
    slot_of = np.empty(N, np.int64)
    core_of[order] = np.arange(N) % NC
    slot_of[order] = np.arange(N) // NC           # 0..12499
    node_of = np.empty((NC, NOWN), np.int64)      # (core, slot) -> node id
    node_of[np.arange(N) % NC, np.arange(N) // NC] = order

    # append self loops as ordinary edges
    loop = np.arange(N, dtype=np.int64)
    src_f = np.concatenate([src, loop])           # [E+N]
    dst_f = np.concatenate([dst, loop])

    ecore = core_of[dst_f]
    eslot = slot_of[dst_f]
    chunk = eslot >> 7
    ldst = eslot & 127
    bank = src_f // BANKROWS
    cell = (chunk * BANKS + bank)

    counts = np.zeros((NC, NCHUNK * BANKS), np.int64)
    for c in range(NC):
        counts[c] = np.bincount(cell[ecore == c], minlength=NCHUNK * BANKS)
    maxcnt = counts.max(axis=0).reshape(NCHUNK, BANKS)
    ntiles_cb = (maxcnt + 127) // 128
    t_ch = ntiles_cb.sum(axis=1)                  # tiles per chunk
    tile_base = np.zeros(NCHUNK + 1, np.int64)
    tile_base[1:] = np.cumsum(t_ch)
    ng_ch = (t_ch + GS - 1) // GS                 # groups per chunk
    group_base = np.zeros(NCHUNK + 1, np.int64)
    group_base[1:] = np.cumsum(ng_ch)
    cell_tile_ofs = np.cumsum(ntiles_cb, axis=1) - ntiles_cb

    # global ordering of edges: (core, chunk, bank), stable
    key = (ecore * (NCHUNK * BANKS) + cell)
    eorder = np.argsort(key, kind="stable")
    key_s = key[eorder]
    starts = np.zeros(NC * NCHUNK * BANKS + 1, np.int64)
    starts[1:] = np.cumsum(np.bincount(key_s, minlength=NC * NCHUNK * BANKS))
    rank = np.arange(src_f.shape[0], dtype=np.int64) - starts[key_s]

    return dict(src_f=src_f, dst_f=dst_f, deg=deg, node_of=node_of,
                ecore=ecore, chunk=chunk, ldst=ldst, bank=bank,
                eorder=eorder, rank=rank,
                maxcnt=maxcnt, ntiles_cb=ntiles_cb, t_ch=t_ch,
                tile_base=tile_base, ng_ch=ng_ch, group_base=group_base,
                cell_tile_ofs=cell_tile_ofs,
                tot_tiles=int(t_ch.sum()), tot_groups=int(ng_ch.sum()),
                tmax=int(t_ch.max()), ngmax=int(ng_ch.max()))


def _host_arrays(plan, edge_attr, attr_f=None):
    """Per-core idx/ldst/attr streams. attr_f = full per-edge attrs [E+N, DE]."""
    tot_tiles, tot_groups = plan["tot_tiles"], plan["tot_groups"]
    idxcols = tot_tiles * 8
    eorder = plan["eorder"]
    tile_base, cto = plan["tile_base"], plan["cell_tile_ofs"]
    group_base = plan["group_base"]

    ecore_o = plan["ecore"][eorder]
    chunk_o = plan["chunk"][eorder]
    ldst_o = plan["ldst"][eorder]
    bank_o = plan["bank"][eorder]
    src_o = plan["src_f"][eorder]
    rank_o = plan["rank"]                 # already in sorted (eorder) order
    attr_o = attr_f[eorder]

    cell_t0 = tile_base[chunk_o] + cto[chunk_o, bank_o]
    tile_abs = cell_t0 + rank_o // 128
    part = rank_o % 128
    ofs16 = cell_t0 * 8 + rank_o // 16
    r16 = rank_o % 16
    tloc = tile_abs - tile_base[chunk_o]
    gcol = (group_base[chunk_o] + tloc // GS) * 128 + part
    krow = 11 * (tloc % GS)

    per_core = []
    for c in range(NC):
        m = ecore_o == c

        i16 = np.zeros((16, idxcols), np.int16)
        i16[r16[m], ofs16[m]] = (src_o[m] - bank_o[m] * BANKROWS).astype(np.int16)
        idx_xl = np.tile(i16, (8, 1))

        sall = np.zeros((128, tot_tiles * 128), BF16)
        sall[part[m], tile_abs[m] * 128 + ldst_o[m]] = 1.0

        attr8 = np.zeros((88, tot_groups * 128), BF16)
        attr8[(krow[m][:, None] + np.arange(DE)[None, :]),
              gcol[m][:, None]] = attr_o[m].astype(BF16)

        per_core.append(dict(idx_xl=idx_xl, sall=sall, attr8=attr8))
    return per_core


# --------------------------------------------------------------------------
# device program (one SPMD NEFF for 8 cores; layout baked from `plan`)
# --------------------------------------------------------------------------

def _build_device(plan):
    dt = mybir.dt
    maxcnt = plan["maxcnt"]
    ntiles_cb = plan["ntiles_cb"]
    t_ch = plan["t_ch"]
    tile_base = plan["tile_base"]
    group_base = plan["group_base"]
    cto = plan["cell_tile_ofs"]
    tot_tiles = plan["tot_tiles"]
    tot_groups = plan["tot_groups"]
    tmax, ngmax = plan["tmax"], plan["ngmax"]
    assert tmax <= 32, f"tmax={tmax}"

    nc = bacc.Bacc("TRN2", target_bir_lowering=False, debug=False,
                   num_devices=NC, num_swdge_queues=4)

    xT = nc.dram_tensor("xT", [128, NTAB], dt.bfloat16, kind="ExternalInput")
    x_ownT = nc.dram_tensor("x_ownT", [128, NPAD], dt.bfloat16, kind="ExternalInput")
    w_cat = nc.dram_tensor("w_cat", [128, 128], dt.bfloat16, kind="ExternalInput")
    w_ebd = nc.dram_tensor("w_ebd", [88, GS * DOUT], dt.bfloat16, kind="ExternalInput")
    att_bc = nc.dram_tensor("att_bc", [128, DOUT], dt.bfloat16, kind="ExternalInput")
    idx_xl = nc.dram_tensor("idx_xl", [128, tot_tiles * 8], dt.int16, kind="ExternalInput")
    sall_d = nc.dram_tensor("sall", [128, tot_tiles * 128], dt.bfloat16, kind="ExternalInput")
    attr8_t = nc.dram_tensor("attr8", [88, tot_groups * 128], dt.bfloat16, kind="ExternalInput")
    out_d = nc.dram_tensor("out", [128, NCHUNK * DOUT], dt.float32, kind="ExternalOutput")

    qn = [0]

    def next_q():
        q = qn[0] & 3
        qn[0] += 1
        return q

    with tile.TileContext(nc) as tc:
        with (
            tc.tile_pool(name="const", bufs=1) as constp,
            tc.tile_pool(name="own", bufs=1) as ownp,
            tc.tile_pool(name="dram", bufs=1, space="DRAM") as dramp,
            tc.tile_pool(name="tload", bufs=3) as tloadp,
            tc.tile_pool(name="stream", bufs=5) as streamp,
            tc.tile_pool(name="gat", bufs=4) as gatp,
            tc.tile_pool(name="sall", bufs=3) as sallp,
            tc.tile_pool(name="sje", bufs=4) as sjep,
            tc.tile_pool(name="work", bufs=6) as workp,
            tc.tile_pool(name="psT", bufs=2, space="PSUM") as psT,
            tc.tile_pool(name="psM", bufs=3, space="PSUM") as psM,
            tc.tile_pool(name="psA", bufs=3, space="PSUM") as psA,
        ):
            # ---------------- constants
            iota_pm = constp.tile([128, 128], dt.int16, tag="iota_pm")
            nc.gpsimd.iota(iota_pm[:], pattern=[[1, 128]], base=0, channel_multiplier=-1)
            iota_pmf = constp.tile([128, 128], dt.float32, tag="iota_pmf")
            nc.vector.tensor_copy(iota_pmf[:], iota_pm[:])
            ident_b = constp.tile([128, 128], dt.bfloat16, tag="ident_b")
            nc.vector.tensor_scalar(out=ident_b[:], in0=iota_pmf[:], scalar1=0.0,
                                    scalar2=None, op0=OP.is_equal)

            wcat_b = constp.tile([128, 128], dt.bfloat16, tag="wcat_b")
            nc.sync.dma_start(wcat_b[:], w_cat[:])
            webd_b = constp.tile([88, GS * DOUT], dt.bfloat16, tag="webd_b")
            nc.sync.dma_start(webd_b[:], w_ebd[:])
            attb_b = constp.tile([128, DOUT], dt.bfloat16, tag="attb_b")
            nc.sync.dma_start(attb_b[:], att_bc[:])

            xr_own = ownp.tile([128, NCHUNK, DOUT], dt.bfloat16, tag="xr_own")
            agg_all = ownp.tile([128, NCHUNK, RHS_W], dt.float32, tag="agg_all")


            C_tab = dramp.tile([NTAB, 128], dt.bfloat16)

            # ---------------- phase 1a: node table C[n] = [xl(n) | xr(n)]
            # xT column (s*2048 + j*128 + p) holds node with table row
            # (s*128 + p)*16 + j, so each slab writes 2048 contiguous elems
            # per partition (fat DMA descriptors).
            for s in range(NSLAB):
                xt_b = tloadp.tile([128, 2048], dt.bfloat16, tag="xt_b")
                nc.sync.dma_start(xt_b[:], xT[:, s * 2048:(s + 1) * 2048])
                cs = tloadp.tile([128, 2048], dt.bfloat16, tag="cs")
                for q in range(4):
                    ps = psM.tile([128, 512], dt.float32, tag="psM")
                    for j in range(4):
                        nc.tensor.matmul(
                            ps[:, j * 128:(j + 1) * 128],
                            lhsT=xt_b[:, (q * 4 + j) * 128:(q * 4 + j + 1) * 128],
                            rhs=wcat_b[:], start=True, stop=True)
                    nc.vector.tensor_copy(cs[:, q * 512:(q + 1) * 512], ps[:])
                # den column baked into the table: row = [xl | 1 | junk]
                nc.vector.memset(
                    cs[:, :].rearrange("p (n f) -> p n f", f=128)[:, :, DOUT:DOUT + 1],
                    1.0)
                nc.sync.dma_start(C_tab[s * 2048:(s + 1) * 2048, :], cs[:])

            # ---------------- phase 1b: xr for own nodes
            for cb in range(0, NCHUNK, 8):
                nch = min(8, NCHUNK - cb)
                xo_b = tloadp.tile([128, 1024], dt.bfloat16, tag="xo_b")
                nc.sync.dma_start(xo_b[:, 0:nch * 128],
                                  x_ownT[:, cb * 128:(cb + nch) * 128])
                ps = psM.tile([128, 512], dt.float32, tag="psM")
                for j in range(nch):
                    nc.tensor.matmul(ps[:, j * 64:(j + 1) * 64],
                                     lhsT=xo_b[:, j * 128:(j + 1) * 128],
                                     rhs=wcat_b[:, 64:128], start=True, stop=True)
                nc.scalar.copy(xr_own[:, cb:cb + nch, :], ps[:, 0:nch * 64])

            # ---------------- phase 2: per-chunk edge pipeline
            for ch in range(NCHUNK):
                T = int(t_ch[ch])
                tb = int(tile_base[ch])
                gb = int(group_base[ch])
                ngr = _cdiv(T, GS)

                idxl = streamp.tile([128, tmax * 8], dt.int16, tag="idxl")
                nc.sync.dma_start(idxl[:, 0:T * 8], idx_xl[:, tb * 8:(tb + T) * 8])
                a8 = streamp.tile([128, ngmax * 128], dt.bfloat16, tag="a8")
                nc.sync.dma_start(a8[0:88, 0:ngr * 128],
                                  attr8_t[:, gb * 128:(gb + ngr) * 128])
                sa = sallp.tile([128, tmax, 128], dt.bfloat16, tag="sa")
                nc.sync.dma_start(sa[:, 0:T, :].rearrange("p t d -> p (t d)"),
                                  sall_d[:, tb * 128:(tb + T) * 128])

                g = gatp.tile([128, tmax, 128], dt.bfloat16, tag="g")
                if ch < 4:  # first uses of the rotating slots: clear stale bits
                    nc.vector.memset(g[:, :, :], 0.0)
                for b in range(BANKS):
                    ncb = int(ntiles_cb[ch, b])
                    if ncb == 0:
                        continue
                    mc = int(maxcnt[ch, b])
                    bofs = int(cto[ch, b])
                    nc.gpsimd.dma_gather(
                        out_ap=g[:, bofs:bofs + ncb, :],
                        in_ap=C_tab[b * BANKROWS:(b + 1) * BANKROWS, :],
                        idxs_ap=idxl[:, bofs * 8:bofs * 8 + _cdiv(mc, 16)],
                        num_idxs=mc, num_idxs_reg=mc, elem_size=128,
                        queue_num=next_q())

                agg = psA.tile([128, RHS_W], dt.float32, tag="psA")
                for gi in range(ngr):
                    t0 = gi * GS
                    ngt = min(GS, T - t0)

                    pst = psT.tile([128, GS * 128], dt.bfloat16, tag="psT")
                    for k in range(ngt):
                        nc.tensor.transpose(pst[:, k * 128:(k + 1) * 128],
                                            in_=sa[:, t0 + k, :], identity=ident_b[:])
                    sje = sjep.tile([128, GS * 128], dt.bfloat16, tag="sje")
                    nc.scalar.copy(sje[:, 0:ngt * 128], pst[:, 0:ngt * 128])

                    # psm = attr @ We (block-diag) + xr[dst] + xl[src]
                    pm = psM.tile([128, 512], dt.float32, tag="psM")
                    nc.tensor.matmul(pm[:, 0:ngt * 64],
                                     lhsT=a8[0:88, gi * 128:(gi + 1) * 128],
                                     rhs=webd_b[:, 0:ngt * 64],
                                     start=True, stop=False)
                    for k in range(ngt):
                        nc.tensor.matmul(pm[:, k * 64:(k + 1) * 64],
                                         lhsT=sje[:, k * 128:(k + 1) * 128],
                                         rhs=xr_own[:, ch, :],
                                         start=False, stop=False)
                    for k in range(ngt):
                        nc.tensor.matmul(pm[:, k * 64:(k + 1) * 64],
                                         lhsT=ident_b[:],
                                         rhs=g[:, t0 + k, 0:DOUT],
                                         start=False, stop=True)

                    mrelu = workp.tile([128, GS, DOUT], dt.bfloat16, tag="mrelu")
                    nc.scalar.activation(
                        mrelu[:, 0:ngt, :].rearrange("p t d -> p (t d)"),
                        pm[:, 0:ngt * 64], AF.Prelu, alpha=NEG_SLOPE)
                    lt = workp.tile([128, GS, DOUT], dt.bfloat16, tag="lt")
                    nc.vector.tensor_tensor(out=lt[:, 0:ngt, :],
                                            in0=mrelu[:, 0:ngt, :],
                                            in1=_bc3(attb_b[:, :], ngt), op=OP.mult)
                    logits = workp.tile([128, GS], dt.float32, tag="logits")
                    nc.vector.tensor_reduce(out=logits[:, 0:ngt], in_=lt[:, 0:ngt, :],
                                            axis=mybir.AxisListType.X, op=OP.add)
                    ex = workp.tile([128, GS], dt.bfloat16, tag="ex")
                    nc.scalar.activation(ex[:, 0:ngt], logits[:, 0:ngt], AF.Exp)

                    # rhs = ex * [xl | 1] (the 1 is baked into table column 64)
                    rhs = workp.tile([128, GS, RHS_W], dt.bfloat16, tag="rhs")
                    nc.vector.tensor_tensor(out=rhs[:, 0:ngt, :],
                                            in0=g[:, t0:t0 + ngt, 0:RHS_W],
                                            in1=_in3(ex[:, 0:ngt], RHS_W), op=OP.mult)

                    for k in range(ngt):
                        nc.tensor.matmul(agg[:], lhsT=sa[:, t0 + k, :],
                                         rhs=rhs[:, k, :],
                                         start=(t0 + k == 0), stop=(t0 + k == T - 1))

                nc.vector.tensor_copy(agg_all[:, ch, :], agg[:])

            # ---------------- epilogue: out = num / den
            rden = ownp.tile([128, NCHUNK], dt.float32, tag="rden")
            nc.vector.reciprocal(rden[:], agg_all[:, :, DOUT])
            o1 = ownp.tile([128, NCHUNK, DOUT], dt.float32, tag="o1")
            nc.vector.tensor_tensor(out=o1[:], in0=agg_all[:, :, 0:DOUT],
                                    in1=_in3(rden[:, :], DOUT), op=OP.mult)
            nc.sync.dma_start(out_d[:, :],
                              o1[:, :, :].rearrange("p t d -> p (t d)"))

    nc.compile()
    return nc


# --------------------------------------------------------------------------
# entry point
# --------------------------------------------------------------------------

def kernel(x, edge_index, edge_attr, W_l, W_r, W_e, att):
    global last_exec_time_ns, last_result
    x = np.asarray(x, np.float32)
    edge_attr = np.asarray(edge_attr, np.float32)
    W_l = np.asarray(W_l, np.float32)
    W_r = np.asarray(W_r, np.float32)
    W_e = np.asarray(W_e, np.float32)
    att = np.asarray(att, np.float32)

    plan = _plan(edge_index)

    # self-loop attrs = mean of incoming edge attrs per destination node
    dst = np.asarray(edge_index[1]).astype(np.int64)
    deg = plan["deg"].astype(np.float32)
    loop_attr = np.empty((N, DE), np.float32)
    for a in range(DE):
        loop_attr[:, a] = np.bincount(dst, weights=edge_attr[:, a], minlength=N)
    loop_attr /= np.maximum(deg, 1.0)[:, None]
    attr_f = np.concatenate([edge_attr, loop_attr], axis=0)

    per_core = _host_arrays(plan, edge_attr, attr_f)

    key = plan["maxcnt"].tobytes() + plan["t_ch"].tobytes()
    if key not in _CACHE:
        _CACHE[key] = _build_device(plan)
    nc = _CACHE[key]

    # xT column (s*2048 + j*128 + p) holds node (s*128 + p)*16 + j
    cidx = np.arange(NTAB)
    node_for_col = ((cidx // 2048) * 128 + (cidx % 128)) * 16 + (cidx % 2048) // 128
    xpad = np.zeros((NTAB, DIN), np.float32)
    xpad[:N] = x
    xT = xpad[node_for_col].T.astype(BF16)

    w_cat = np.concatenate([W_l, W_r], axis=1).astype(BF16)
    w_ebd = np.zeros((88, GS * DOUT), np.float32)
    for k in range(GS):
        w_ebd[11 * k:11 * (k + 1), 64 * k:64 * (k + 1)] = W_e
    w_ebd = w_ebd.astype(BF16)
    att_bc = np.tile(att[None, :], (128, 1)).astype(BF16)

    node_of = plan["node_of"]
    in_maps = []
    for c in range(NC):
        x_ownT = np.zeros((128, NPAD), BF16)
        x_ownT[:, :NOWN] = x[node_of[c]].T.astype(BF16)
        pc = per_core[c]
        in_maps.append({
            "xT": xT, "x_ownT": x_ownT, "w_cat": w_cat, "w_ebd": w_ebd,
            "att_bc": att_bc, "idx_xl": pc["idx_xl"],
            "sall": pc["sall"], "attr8": pc["attr8"],
        })

    try:
        res = run_bass_kernel_spmd(nc, in_maps, core_ids=list(range(NC)), trace=True)
        last_exec_time_ns = res.exec_time_ns
    except Exception:
        res = run_bass_kernel_spmd(nc, in_maps, core_ids=list(range(NC)), trace=False)
        last_exec_time_ns = None
    last_result = res

    out = np.zeros((N, DOUT), np.float32)
    for c in range(NC):
        r = res.results[c]["out"].reshape(128, NCHUNK, DOUT)
        r = r.transpose(1, 0, 2).reshape(NPAD, DOUT)
        out[node_of[c]] = r[:NOWN]
    return out


# revision 17
# speedup vs baseline: 1.1032x; 1.1032x over previous
"""GATv2Conv (heads=1, edge_dim=11, add_self_loops fill='mean') on 8 Trainium2 cores.

Sharding: nodes dealt to cores by degree rank (core = rank % 8), so per-cell
edge counts are nearly identical across cores and the shared-NEFF padding is
minimal. Edges grouped per (dst-chunk, src-bank) cell; per-edge xl[src] rows
come from one dma_gather per cell out of an on-device bf16 table C[n]=[xl|xr].
Self-loops are folded in as ordinary edges (attr = host-computed mean).

Per chunk, tiles are processed in groups of 8: one-hot S built by a single DVE
compare, transposed on the PE (batched PSUM->SBUF copy), messages assembled in
PSUM via bf16 matmuls only (block-diagonal attr@We + S^T@xr + identity@xl),
softmax weights applied, and segment sums done by S one-hot matmuls.

All fp32 has been removed from the DMA and matmul paths (bf16 end to end,
fp32 only in PSUM accumulation and the final divide). Host work is layout
only: sharding, grouping, index packing, casts, and the self-loop attr mean.
"""

import sys

sys.path.insert(0, "/opt/trn_rl_repo")

import numpy as np
import ml_dtypes

import concourse.bass as bass
import concourse.bacc as bacc
import concourse.tile as tile
import concourse.mybir as mybir
from concourse.bass_utils import run_bass_kernel_spmd

BF16 = ml_dtypes.bfloat16
AF = mybir.ActivationFunctionType
OP = mybir.AluOpType

N, E, DIN, DOUT, DE = 1000_00, 1000_000, 128, 64, 11
NEG_SLOPE = 0.2
NC = 8
NOWN = N // NC                    # 12500
NCHUNK = 98                       # ceil(12500/128)
NPAD = NCHUNK * 128               # 12544
BANKS = 4
BANKROWS = 25024                  # 4*25024 = 100096 >= N; < 2^15 for int16 idx
NSLAB = 49                        # table build slabs of 2048 rows
NTAB = NSLAB * 2048               # 100352 >= 100096
GS = 8                            # tiles per processing group
RHS_W = 65                        # [ex | ex*xl]

last_exec_time_ns = None
last_result = None
_CACHE = {}


def _cdiv(a, b):
    return -(-a // b)


def _bc3(ap2, mid):
    """[P, F] AP -> [P, mid, F] AP broadcast along a new middle dim."""
    return bass.AP(ap2.tensor, ap2.offset, [ap2.ap[0], [0, mid], ap2.ap[1]])


def _in3(ap2, inner):
    """[P, T] AP -> [P, T, inner] AP broadcast along a new inner dim."""
    return bass.AP(ap2.tensor, ap2.offset, [ap2.ap[0], ap2.ap[1], [0, inner]])


# --------------------------------------------------------------------------
# host-side layout (index manipulation only)
# --------------------------------------------------------------------------

def _plan(edge_index):
    src = np.asarray(edge_index[0]).astype(np.int64)
    dst = np.asarray(edge_index[1]).astype(np.int64)
    deg = np.bincount(dst, minlength=N).astype(np.int64)

    # deal nodes to cores by degree rank: nearly equal per-cell counts
    order = np.argsort(deg, kind="stable")        # node ids, ascending degree
    core_of = np.empty(N, np.int64)
    slot_of = np.empty(N, np.int64)
    core_of[order] = np.arange(N) % NC
    slot_of[order] = np.arange(N) // NC           # 0..12499
    node_of = np.empty((NC, NOWN), np.int64)      # (core, slot) -> node id
    node_of[np.arange(N) % NC, np.arange(N) // NC] = order

    # append self loops as ordinary edges
    loop = np.arange(N, dtype=np.int64)
    src_f = np.concatenate([src, loop])           # [E+N]
    dst_f = np.concatenate([dst, loop])

    ecore = core_of[dst_f]
    eslot = slot_of[dst_f]
    chunk = eslot >> 7
    ldst = eslot & 127
    bank = src_f // BANKROWS
    cell = (chunk * BANKS + bank)

    counts = np.zeros((NC, NCHUNK * BANKS), np.int64)
    for c in range(NC):
        counts[c] = np.bincount(cell[ecore == c], minlength=NCHUNK * BANKS)
    maxcnt = counts.max(axis=0).reshape(NCHUNK, BANKS)
    ntiles_cb = (maxcnt + 127) // 128

    # chunk pairs: tile order within pair p = [b0: ch 2p, 2p+1 | b1: ... | b3]
    NP = NCHUNK // 2
    nt_pb = ntiles_cb[0::2] + ntiles_cb[1::2]     # [NP, BANKS] tiles per (pair, bank)
    t_pair = nt_pb.sum(axis=1)                    # tiles per pair
    pair_base = np.zeros(NP + 1, np.int64)
    pair_base[1:] = np.cumsum(t_pair)
    bank_ofs = np.cumsum(nt_pb, axis=1) - nt_pb   # [NP, BANKS]
    # cell (ch, b) tile offset within its pair
    cell_tile_ofs = np.empty((NCHUNK, BANKS), np.int64)
    cell_tile_ofs[0::2] = bank_ofs
    cell_tile_ofs[1::2] = bank_ofs + ntiles_cb[0::2]
    tile_base = np.repeat(pair_base[:-1], 2)      # [NCHUNK]: pair start per chunk
    ng_pair = (t_pair + GS - 1) // GS             # groups per pair
    group_base = np.zeros(NP + 1, np.int64)
    group_base[1:] = np.cumsum(ng_pair)
    # per-tile chunk-within-pair (0/1) and per-chunk first/last tile flags
    tot_tiles = int(t_pair.sum())
    chloc_of_tile = np.zeros(tot_tiles, np.int64)
    first_of_chunk = np.zeros(NCHUNK, np.int64)
    last_of_chunk = np.zeros(NCHUNK, np.int64)
    for p in range(NP):
        segs = []
        for b in range(BANKS):
            for cl in range(2):
                nt = int(ntiles_cb[2 * p + cl, b])
                segs += [cl] * nt
        segs = np.array(segs, np.int64)
        chloc_of_tile[pair_base[p]:pair_base[p + 1]] = segs
        for cl in range(2):
            w = np.where(segs == cl)[0]
            first_of_chunk[2 * p + cl] = pair_base[p] + w[0]
            last_of_chunk[2 * p + cl] = pair_base[p] + w[-1]

    # global ordering of edges: (core, chunk, bank), stable
    key = (ecore * (NCHUNK * BANKS) + cell)
    eorder = np.argsort(key, kind="stable")
    key_s = key[eorder]
    starts = np.zeros(NC * NCHUNK * BANKS + 1, np.int64)
    starts[1:] = np.cumsum(np.bincount(key_s, minlength=NC * NCHUNK * BANKS))
    rank = np.arange(src_f.shape[0], dtype=np.int64) - starts[key_s]

    return dict(src_f=src_f, dst_f=dst_f, deg=deg, node_of=node_of,
                ecore=ecore, chunk=chunk, ldst=ldst, bank=bank,
                eorder=eorder, rank=rank,
                maxcnt=maxcnt, ntiles_cb=ntiles_cb, nt_pb=nt_pb,
                t_pair=t_pair, pair_base=pair_base, bank_ofs=bank_ofs,
                tile_base=tile_base, ng_pair=ng_pair, group_base=group_base,
                cell_tile_ofs=cell_tile_ofs, chloc_of_tile=chloc_of_tile,
                first_of_chunk=first_of_chunk, last_of_chunk=last_of_chunk,
                tot_tiles=tot_tiles, tot_groups=int(ng_pair.sum()),
                tmax=int(t_pair.max()), ngmax=int(ng_pair.max()))


def _host_arrays(plan, edge_attr, attr_f=None):
    """Per-core idx/ldst/attr streams. attr_f = full per-edge attrs [E+N, DE]."""
    tot_tiles, tot_groups = plan["tot_tiles"], plan["tot_groups"]
    idxcols = tot_tiles * 8
    eorder = plan["eorder"]
    tile_base, cto = plan["tile_base"], plan["cell_tile_ofs"]
    group_base = plan["group_base"]

    ecore_o = plan["ecore"][eorder]
    chunk_o = plan["chunk"][eorder]
    ldst_o = plan["ldst"][eorder]
    bank_o = plan["bank"][eorder]
    src_o = plan["src_f"][eorder]
    rank_o = plan["rank"]                 # already in sorted (eorder) order
    attr_o = attr_f[eorder]

    cell_t0 = tile_base[chunk_o] + cto[chunk_o, bank_o]
    tile_abs = cell_t0 + rank_o // 128
    part = rank_o % 128
    ofs16 = cell_t0 * 8 + rank_o // 16
    r16 = rank_o % 16
    tloc = tile_abs - tile_base[chunk_o]          # tile within pair
    gcol = (group_base[chunk_o // 2] + tloc // GS) * 128 + part
    krow = 11 * (tloc % GS)

    per_core = []
    for c in range(NC):
        m = ecore_o == c

        i16 = np.zeros((16, idxcols), np.int16)
        i16[r16[m], ofs16[m]] = (src_o[m] - bank_o[m] * BANKROWS).astype(np.int16)
        idx_xl = np.tile(i16, (8, 1))

        sall = np.zeros((128, tot_tiles * 128), BF16)
        sall[part[m], tile_abs[m] * 128 + ldst_o[m]] = 1.0

        attr8 = np.zeros((88, tot_groups * 128), BF16)
        attr8[(krow[m][:, None] + np.arange(DE)[None, :]),
              gcol[m][:, None]] = attr_o[m].astype(BF16)

        per_core.append(dict(idx_xl=idx_xl, sall=sall, attr8=attr8))
    return per_core


# --------------------------------------------------------------------------
# device program (one SPMD NEFF for 8 cores; layout baked from `plan`)
# --------------------------------------------------------------------------

def _build_device(plan):
    dt = mybir.dt
    maxcnt = plan["maxcnt"]
    ntiles_cb = plan["ntiles_cb"]
    t_ch = plan["t_ch"]
    tile_base = plan["tile_base"]
    group_base = plan["group_base"]
    cto = plan["cell_tile_ofs"]
    tot_tiles = plan["tot_tiles"]
    tot_groups = plan["tot_groups"]
    tmax, ngmax = plan["tmax"], plan["ngmax"]
    assert tmax <= 32, f"tmax={tmax}"

    nc = bacc.Bacc("TRN2", target_bir_lowering=False, debug=False,
                   num_devices=NC, num_swdge_queues=4)

    xT = nc.dram_tensor("xT", [128, NTAB], dt.bfloat16, kind="ExternalInput")
    x_ownT = nc.dram_tensor("x_ownT", [128, NPAD], dt.bfloat16, kind="ExternalInput")
    w_cat = nc.dram_tensor("w_cat", [128, 128], dt.bfloat16, kind="ExternalInput")
    w_ebd = nc.dram_tensor("w_ebd", [88, GS * DOUT], dt.bfloat16, kind="ExternalInput")
    att_bc = nc.dram_tensor("att_bc", [128, DOUT], dt.bfloat16, kind="ExternalInput")
    idx_xl = nc.dram_tensor("idx_xl", [128, tot_tiles * 8], dt.int16, kind="ExternalInput")
    sall_d = nc.dram_tensor("sall", [128, tot_tiles * 128], dt.bfloat16, kind="ExternalInput")
    attr8_t = nc.dram_tensor("attr8", [88, tot_groups * 128], dt.bfloat16, kind="ExternalInput")
    out_d = nc.dram_tensor("out", [128, NCHUNK * DOUT], dt.float32, kind="ExternalOutput")

    qn = [0]

    def next_q():
        q = qn[0] & 3
        qn[0] += 1
        return q

    with tile.TileContext(nc) as tc:
        with (
            tc.tile_pool(name="const", bufs=1) as constp,
            tc.tile_pool(name="own", bufs=1) as ownp,
            tc.tile_pool(name="dram", bufs=1, space="DRAM") as dramp,
            tc.tile_pool(name="tload", bufs=3) as tloadp,
            tc.tile_pool(name="stream", bufs=6) as streamp,
            tc.tile_pool(name="gat", bufs=6) as gatp,
            tc.tile_pool(name="sall", bufs=4) as sallp,
            tc.tile_pool(name="sje", bufs=4) as sjep,
            tc.tile_pool(name="work", bufs=6) as workp,
            tc.tile_pool(name="psT", bufs=2, space="PSUM") as psT,
            tc.tile_pool(name="psM", bufs=3, space="PSUM") as psM,
            tc.tile_pool(name="psA", bufs=3, space="PSUM") as psA,
        ):
            # ---------------- constants
            iota_pm = constp.tile([128, 128], dt.int16, tag="iota_pm")
            nc.gpsimd.iota(iota_pm[:], pattern=[[1, 128]], base=0, channel_multiplier=-1)
            iota_pmf = constp.tile([128, 128], dt.float32, tag="iota_pmf")
            nc.vector.tensor_copy(iota_pmf[:], iota_pm[:])
            ident_b = constp.tile([128, 128], dt.bfloat16, tag="ident_b")
            nc.vector.tensor_scalar(out=ident_b[:], in0=iota_pmf[:], scalar1=0.0,
                                    scalar2=None, op0=OP.is_equal)

            wcat_b = constp.tile([128, 128], dt.bfloat16, tag="wcat_b")
            nc.sync.dma_start(wcat_b[:], w_cat[:])
            webd_b = constp.tile([88, GS * DOUT], dt.bfloat16, tag="webd_b")
            nc.sync.dma_start(webd_b[:], w_ebd[:])
            attb_b = constp.tile([128, DOUT], dt.bfloat16, tag="attb_b")
            nc.sync.dma_start(attb_b[:], att_bc[:])

            xr_own = ownp.tile([128, NCHUNK, DOUT], dt.bfloat16, tag="xr_own")
            agg_all = ownp.tile([128, NCHUNK, RHS_W], dt.float32, tag="agg_all")


            C_tab = dramp.tile([NTAB, 128], dt.bfloat16)

            # ---------------- phase 1a: node table C[n] = [xl(n) | xr(n)]
            # xT column (s*2048 + j*128 + p) holds node with table row
            # (s*128 + p)*16 + j, so each slab writes 2048 contiguous elems
            # per partition (fat DMA descriptors).
            for s in range(NSLAB):
                xt_b = tloadp.tile([128, 2048], dt.bfloat16, tag="xt_b")
                nc.sync.dma_start(xt_b[:], xT[:, s * 2048:(s + 1) * 2048])
                cs = tloadp.tile([128, 2048], dt.bfloat16, tag="cs")
                for q in range(4):
                    ps = psM.tile([128, 512], dt.float32, tag="psM")
                    for j in range(4):
                        nc.tensor.matmul(
                            ps[:, j * 128:(j + 1) * 128],
                            lhsT=xt_b[:, (q * 4 + j) * 128:(q * 4 + j + 1) * 128],
                            rhs=wcat_b[:], start=True, stop=True)
                    nc.vector.tensor_copy(cs[:, q * 512:(q + 1) * 512], ps[:])
                # den column baked into the table: row = [xl | 1 | junk]
                nc.vector.memset(
                    cs[:, :].rearrange("p (n f) -> p n f", f=128)[:, :, DOUT:DOUT + 1],
                    1.0)
                nc.sync.dma_start(C_tab[s * 2048:(s + 1) * 2048, :], cs[:])

            # ---------------- phase 1b: xr for own nodes
            for cb in range(0, NCHUNK, 8):
                nch = min(8, NCHUNK - cb)
                xo_b = tloadp.tile([128, 1024], dt.bfloat16, tag="xo_b")
                nc.sync.dma_start(xo_b[:, 0:nch * 128],
                                  x_ownT[:, cb * 128:(cb + nch) * 128])
                ps = psM.tile([128, 512], dt.float32, tag="psM")
                for j in range(nch):
                    nc.tensor.matmul(ps[:, j * 64:(j + 1) * 64],
                                     lhsT=xo_b[:, j * 128:(j + 1) * 128],
                                     rhs=wcat_b[:, 64:128], start=True, stop=True)
                nc.scalar.copy(xr_own[:, cb:cb + nch, :], ps[:, 0:nch * 64])

            # ---------------- phase 2: per-chunk edge pipeline
            for ch in range(NCHUNK):
                T = int(t_ch[ch])
                tb = int(tile_base[ch])
                gb = int(group_base[ch])
                ngr = _cdiv(T, GS)

                idxl = streamp.tile([128, tmax * 8], dt.int16, tag="idxl")
                nc.sync.dma_start(idxl[:, 0:T * 8], idx_xl[:, tb * 8:(tb + T) * 8])
                a8 = streamp.tile([128, ngmax * 128], dt.bfloat16, tag="a8")
                nc.sync.dma_start(a8[0:88, 0:ngr * 128],
                                  attr8_t[:, gb * 128:(gb + ngr) * 128])
                sa = sallp.tile([128, tmax, 128], dt.bfloat16, tag="sa")
                nc.sync.dma_start(sa[:, 0:T, :].rearrange("p t d -> p (t d)"),
                                  sall_d[:, tb * 128:(tb + T) * 128])

                g = gatp.tile([128, tmax, 128], dt.bfloat16, tag="g")
                if ch < 4:  # first uses of the rotating slots: clear stale bits
                    nc.vector.memset(g[:, :, :], 0.0)
                for b in range(BANKS):
                    ncb = int(ntiles_cb[ch, b])
                    if ncb == 0:
                        continue
                    mc = int(maxcnt[ch, b])
                    bofs = int(cto[ch, b])
                    nc.gpsimd.dma_gather(
                        out_ap=g[:, bofs:bofs + ncb, :],
                        in_ap=C_tab[b * BANKROWS:(b + 1) * BANKROWS, :],
                        idxs_ap=idxl[:, bofs * 8:bofs * 8 + _cdiv(mc, 16)],
                        num_idxs=mc, num_idxs_reg=mc, elem_size=128,
                        queue_num=next_q())

                agg = psA.tile([128, RHS_W], dt.float32, tag="psA")
                for gi in range(ngr):
                    t0 = gi * GS
                    ngt = min(GS, T - t0)

                    pst = psT.tile([128, GS * 128], dt.bfloat16, tag="psT")
                    for k in range(ngt):
                        nc.tensor.transpose(pst[:, k * 128:(k + 1) * 128],
                                            in_=sa[:, t0 + k, :], identity=ident_b[:])
                    sje = sjep.tile([128, GS * 128], dt.bfloat16, tag="sje")
                    nc.scalar.copy(sje[:, 0:ngt * 128], pst[:, 0:ngt * 128])

                    # psm = attr @ We (block-diag) + xr[dst] + xl[src]
                    pm = psM.tile([128, 512], dt.float32, tag="psM")
                    nc.tensor.matmul(pm[:, 0:ngt * 64],
                                     lhsT=a8[0:88, gi * 128:(gi + 1) * 128],
                                     rhs=webd_b[:, 0:ngt * 64],
                                     start=True, stop=False)
                    for k in range(ngt):
                        nc.tensor.matmul(pm[:, k * 64:(k + 1) * 64],
                                         lhsT=sje[:, k * 128:(k + 1) * 128],
                                         rhs=xr_own[:, ch, :],
                                         start=False, stop=False)
                    for k in range(ngt):
                        nc.tensor.matmul(pm[:, k * 64:(k + 1) * 64],
                                         lhsT=ident_b[:],
                                         rhs=g[:, t0 + k, 0:DOUT],
                                         start=False, stop=True)

                    mrelu = workp.tile([128, GS, DOUT], dt.bfloat16, tag="mrelu")
                    nc.scalar.activation(
                        mrelu[:, 0:ngt, :].rearrange("p t d -> p (t d)"),
                        pm[:, 0:ngt * 64], AF.Prelu, alpha=NEG_SLOPE)
                    lt = workp.tile([128, GS, DOUT], dt.bfloat16, tag="lt")
                    nc.vector.tensor_tensor(out=lt[:, 0:ngt, :],
                                            in0=mrelu[:, 0:ngt, :],
                                            in1=_bc3(attb_b[:, :], ngt), op=OP.mult)
                    logits = workp.tile([128, GS], dt.float32, tag="logits")
                    nc.vector.tensor_reduce(out=logits[:, 0:ngt], in_=lt[:, 0:ngt, :],
                                            axis=mybir.AxisListType.X, op=OP.add)
                    ex = workp.tile([128, GS], dt.bfloat16, tag="ex")
                    nc.scalar.activation(ex[:, 0:ngt], logits[:, 0:ngt], AF.Exp)

                    # rhs = ex * [xl | 1] (the 1 is baked into table column 64)
                    rhs = workp.tile([128, GS, RHS_W], dt.bfloat16, tag="rhs")
                    nc.vector.tensor_tensor(out=rhs[:, 0:ngt, :],
                                            in0=g[:, t0:t0 + ngt, 0:RHS_W],
                                            in1=_in3(ex[:, 0:ngt], RHS_W), op=OP.mult)

                    for k in range(ngt):
                        nc.tensor.matmul(agg[:], lhsT=sa[:, t0 + k, :],
                                         rhs=rhs[:, k, :],
                                         start=(t0 + k == 0), stop=(t0 + k == T - 1))

                nc.vector.tensor_copy(agg_all[:, ch, :], agg[:])

            # ---------------- epilogue: out = num / den
            rden = ownp.tile([128, NCHUNK], dt.float32, tag="rden")
            nc.vector.reciprocal(rden[:], agg_all[:, :, DOUT])
            o1 = ownp.tile([128, NCHUNK, DOUT], dt.float32, tag="o1")
            nc.vector.tensor_tensor(out=o1[:], in0=agg_all[:, :, 0:DOUT],
                                    in1=_in3(rden[:, :], DOUT), op=OP.mult)
            nc.sync.dma_start(out_d[:, :],
                              o1[:, :, :].rearrange("p t d -> p (t d)"))

    nc.compile()
    return nc


# --------------------------------------------------------------------------
# entry point
# --------------------------------------------------------------------------

def kernel(x, edge_index, edge_attr, W_l, W_r, W_e, att):
    global last_exec_time_ns, last_result
    x = np.asarray(x, np.float32)
    edge_attr = np.asarray(edge_attr, np.float32)
    W_l = np.asarray(W_l, np.float32)
    W_r = np.asarray(W_r, np.float32)
    W_e = np.asarray(W_e, np.float32)
    att = np.asarray(att, np.float32)

    plan = _plan(edge_index)

    # self-loop attrs = mean of incoming edge attrs per destination node
    dst = np.asarray(edge_index[1]).astype(np.int64)
    deg = plan["deg"].astype(np.float32)
    loop_attr = np.empty((N, DE), np.float32)
    for a in range(DE):
        loop_attr[:, a] = np.bincount(dst, weights=edge_attr[:, a], minlength=N)
    loop_attr /= np.maximum(deg, 1.0)[:, None]
    attr_f = np.concatenate([edge_attr, loop_attr], axis=0)

    per_core = _host_arrays(plan, edge_attr, attr_f)

    key = plan["maxcnt"].tobytes() + plan["t_ch"].tobytes()
    if key not in _CACHE:
        _CACHE[key] = _build_device(plan)
    nc = _CACHE[key]

    # xT column (s*2048 + j*128 + p) holds node (s*128 + p)*16 + j
    cidx = np.arange(NTAB)
    node_for_col = ((cidx // 2048) * 128 + (cidx % 128)) * 16 + (cidx % 2048) // 128
    xpad = np.zeros((NTAB, DIN), np.float32)
    xpad[:N] = x
    xT = xpad[node_for_col].T.astype(BF16)

    w_cat = np.concatenate([W_l, W_r], axis=1).astype(BF16)
    w_ebd = np.zeros((88, GS * DOUT), np.float32)
    for k in range(GS):
        w_ebd[11 * k:11 * (k + 1), 64 * k:64 * (k + 1)] = W_e
    w_ebd = w_ebd.astype(BF16)
    att_bc = np.tile(att[None, :], (128, 1)).astype(BF16)

    node_of = plan["node_of"]
    in_maps = []
    for c in range(NC):
        x_ownT = np.zeros((128, NPAD), BF16)
        x_ownT[:, :NOWN] = x[node_of[c]].T.astype(BF16)
        pc = per_core[c]
        in_maps.append({
            "xT": xT, "x_ownT": x_ownT, "w_cat": w_cat, "w_ebd": w_ebd,
            "att_bc": att_bc, "idx_xl": pc["idx_xl"],
            "sall": pc["sall"], "attr8": pc["attr8"],
        })

    try:
        res = run_bass_kernel_spmd(nc, in_maps, core_ids=list(range(NC)), trace=True)
        last_exec_time_ns = res.exec_time_ns
    except Exception:
        res = run_bass_kernel_spmd(nc, in_maps, core_ids=list(range(NC)), trace=False)
        last_exec_time_ns = None
    last_result = res

    out = np.zeros((N, DOUT), np.float32)
    for c in range(NC):
        r = res.results[c]["out"].reshape(128, NCHUNK, DOUT)
        r = r.transpose(1, 0, 2).reshape(NPAD, DOUT)
        out[node_of[c]] = r[:NOWN]
    return out


# revision 24
# speedup vs baseline: 1.1528x; 1.0449x over previous
"""GATv2Conv (heads=1, edge_dim=11, add_self_loops fill='mean') on 8 Trainium2 cores.

Sharding: nodes dealt to cores by degree rank (core = rank % 8), so per-cell
edge counts are nearly identical across cores and the shared-NEFF padding is
minimal. Edges grouped per (dst-chunk, src-bank) cell; per-edge xl[src] rows
come from one dma_gather per cell out of an on-device bf16 table C[n]=[xl|xr].
Self-loops are folded in as ordinary edges (attr = host-computed mean).

Per chunk, tiles are processed in groups of 8: one-hot S built by a single DVE
compare, transposed on the PE (batched PSUM->SBUF copy), messages assembled in
PSUM via bf16 matmuls only (block-diagonal attr@We + S^T@xr + identity@xl),
softmax weights applied, and segment sums done by S one-hot matmuls.

All fp32 has been removed from the DMA and matmul paths (bf16 end to end,
fp32 only in PSUM accumulation and the final divide). Host work is layout
only: sharding, grouping, index packing, casts, and the self-loop attr mean.
"""

import sys

sys.path.insert(0, "/opt/trn_rl_repo")

import numpy as np
import ml_dtypes

import concourse.bass as bass
import concourse.bacc as bacc
import concourse.tile as tile
import concourse.mybir as mybir
from concourse.bass_utils import run_bass_kernel_spmd

BF16 = ml_dtypes.bfloat16
AF = mybir.ActivationFunctionType
OP = mybir.AluOpType

N, E, DIN, DOUT, DE = 1000_00, 1000_000, 128, 64, 11
NEG_SLOPE = 0.2
NC = 8
NOWN = N // NC                    # 12500
NCHUNK = 98                       # ceil(12500/128)
NPAD = NCHUNK * 128               # 12544
BANKS = 4
BANKROWS = 25024                  # 4*25024 = 100096 >= N; < 2^15 for int16 idx
NSLAB = 49                        # table build slabs of 2048 rows
NTAB = NSLAB * 2048               # 100352 >= 100096
GS = 8                            # tiles per processing group
RHS_W = 65                        # [ex | ex*xl]

last_exec_time_ns = None
last_result = None
_CACHE = {}


def _cdiv(a, b):
    return -(-a // b)


def _bc3(ap2, mid):
    """[P, F] AP -> [P, mid, F] AP broadcast along a new middle dim."""
    return bass.AP(ap2.tensor, ap2.offset, [ap2.ap[0], [0, mid], ap2.ap[1]])


def _in3(ap2, inner):
    """[P, T] AP -> [P, T, inner] AP broadcast along a new inner dim."""
    return bass.AP(ap2.tensor, ap2.offset, [ap2.ap[0], ap2.ap[1], [0, inner]])


# --------------------------------------------------------------------------
# host-side layout (index manipulation only)
# --------------------------------------------------------------------------

def _plan(edge_index):
    src = np.asarray(edge_index[0]).astype(np.int64)
    dst = np.asarray(edge_index[1]).astype(np.int64)
    deg = np.bincount(dst, minlength=N).astype(np.int64)

    # deal nodes to cores by degree rank: nearly equal per-cell counts
    order = np.argsort(deg, kind="stable")        # node ids, ascending degree
    core_of = np.empty(N, np.int64)
    slot_of = np.empty(N, np.int64)
    core_of[order] = np.arange(N) % NC
    slot_of[order] = np.arange(N) // NC           # 0..12499
    node_of = np.empty((NC, NOWN), np.int64)      # (core, slot) -> node id
    node_of[np.arange(N) % NC, np.arange(N) // NC] = order

    # append self loops as ordinary edges
    loop = np.arange(N, dtype=np.int64)
    src_f = np.concatenate([src, loop])           # [E+N]
    dst_f = np.concatenate([dst, loop])

    ecore = core_of[dst_f]
    eslot = slot_of[dst_f]
    chunk = eslot >> 7
    ldst = eslot & 127
    bank = src_f // BANKROWS
    cell = (chunk * BANKS + bank)

    counts = np.zeros((NC, NCHUNK * BANKS), np.int64)
    for c in range(NC):
        counts[c] = np.bincount(cell[ecore == c], minlength=NCHUNK * BANKS)
    maxcnt = counts.max(axis=0).reshape(NCHUNK, BANKS)
    ntiles_cb = (maxcnt + 127) // 128
    t_ch = ntiles_cb.sum(axis=1)                  # tiles per chunk
    tile_base = np.zeros(NCHUNK + 1, np.int64)
    tile_base[1:] = np.cumsum(t_ch)
    ng_ch = (t_ch + GS - 1) // GS                 # groups per chunk
    group_base = np.zeros(NCHUNK + 1, np.int64)
    group_base[1:] = np.cumsum(ng_ch)
    cell_tile_ofs = np.cumsum(ntiles_cb, axis=1) - ntiles_cb

    # global ordering of edges: (core, chunk, bank), stable
    key = (ecore * (NCHUNK * BANKS) + cell)
    eorder = np.argsort(key, kind="stable")
    key_s = key[eorder]
    starts = np.zeros(NC * NCHUNK * BANKS + 1, np.int64)
    starts[1:] = np.cumsum(np.bincount(key_s, minlength=NC * NCHUNK * BANKS))
    rank = np.arange(src_f.shape[0], dtype=np.int64) - starts[key_s]

    return dict(src_f=src_f, dst_f=dst_f, deg=deg, node_of=node_of,
                ecore=ecore, chunk=chunk, ldst=ldst, bank=bank,
                eorder=eorder, rank=rank,
                maxcnt=maxcnt, ntiles_cb=ntiles_cb, t_ch=t_ch,
                tile_base=tile_base, ng_ch=ng_ch, group_base=group_base,
                cell_tile_ofs=cell_tile_ofs,
                tot_tiles=int(t_ch.sum()), tot_groups=int(ng_ch.sum()),
                tmax=int(t_ch.max()), ngmax=int(ng_ch.max()))


def _host_arrays(plan, edge_attr, attr_f=None):
    """Per-core idx/ldst/attr streams. attr_f = full per-edge attrs [E+N, DE]."""
    tot_tiles, tot_groups = plan["tot_tiles"], plan["tot_groups"]
    idxcols = tot_tiles * 8
    eorder = plan["eorder"]
    tile_base, cto = plan["tile_base"], plan["cell_tile_ofs"]
    group_base = plan["group_base"]

    ecore_o = plan["ecore"][eorder]
    chunk_o = plan["chunk"][eorder]
    ldst_o = plan["ldst"][eorder]
    bank_o = plan["bank"][eorder]
    src_o = plan["src_f"][eorder]
    rank_o = plan["rank"]                 # already in sorted (eorder) order
    attr_o = attr_f[eorder]

    cell_t0 = tile_base[chunk_o] + cto[chunk_o, bank_o]
    tile_abs = cell_t0 + rank_o // 128
    part = rank_o % 128
    ofs16 = cell_t0 * 8 + rank_o // 16
    r16 = rank_o % 16
    tloc = tile_abs - tile_base[chunk_o]
    gcol = (group_base[chunk_o] + tloc // GS) * 128 + part
    krow = 11 * (tloc % GS)

    per_core = []
    for c in range(NC):
        m = ecore_o == c

        i16 = np.zeros((16, idxcols), np.int16)
        i16[r16[m], ofs16[m]] = (src_o[m] - bank_o[m] * BANKROWS).astype(np.int16)
        idx_xl = np.tile(i16, (8, 1))

        sall = np.zeros((128, tot_tiles * 128), BF16)
        sall[part[m], tile_abs[m] * 128 + ldst_o[m]] = 1.0

        attr8 = np.zeros((88, tot_groups * 128), BF16)
        attr8[(krow[m][:, None] + np.arange(DE)[None, :]),
              gcol[m][:, None]] = attr_o[m].astype(BF16)

        per_core.append(dict(idx_xl=idx_xl, sall=sall, attr8=attr8))
    return per_core


# --------------------------------------------------------------------------
# device program (one SPMD NEFF for 8 cores; layout baked from `plan`)
# --------------------------------------------------------------------------

def _build_device(plan):
    dt = mybir.dt
    maxcnt = plan["maxcnt"]
    ntiles_cb = plan["ntiles_cb"]
    t_ch = plan["t_ch"]
    tile_base = plan["tile_base"]
    group_base = plan["group_base"]
    cto = plan["cell_tile_ofs"]
    tot_tiles = plan["tot_tiles"]
    tot_groups = plan["tot_groups"]
    tmax, ngmax = plan["tmax"], plan["ngmax"]
    assert tmax <= 32, f"tmax={tmax}"

    nc = bacc.Bacc("TRN2", target_bir_lowering=False, debug=False,
                   num_devices=NC, num_swdge_queues=4)

    xT = nc.dram_tensor("xT", [128, NTAB], dt.bfloat16, kind="ExternalInput")
    x_ownT = nc.dram_tensor("x_ownT", [128, NPAD], dt.bfloat16, kind="ExternalInput")
    w_cat = nc.dram_tensor("w_cat", [128, 128], dt.bfloat16, kind="ExternalInput")
    w_ebd = nc.dram_tensor("w_ebd", [88, GS * DOUT], dt.bfloat16, kind="ExternalInput")
    att_bc = nc.dram_tensor("att_bc", [128, DOUT], dt.bfloat16, kind="ExternalInput")
    idx_xl = nc.dram_tensor("idx_xl", [128, tot_tiles * 8], dt.int16, kind="ExternalInput")
    sall_d = nc.dram_tensor("sall", [128, tot_tiles * 128], dt.bfloat16, kind="ExternalInput")
    attr8_t = nc.dram_tensor("attr8", [88, tot_groups * 128], dt.bfloat16, kind="ExternalInput")
    out_d = nc.dram_tensor("out", [128, NCHUNK * DOUT], dt.float32, kind="ExternalOutput")

    qn = [0]

    def next_q():
        q = qn[0] & 3
        qn[0] += 1
        return q

    with tile.TileContext(nc) as tc:
        with (
            tc.tile_pool(name="const", bufs=1) as constp,
            tc.tile_pool(name="own", bufs=1) as ownp,
            tc.tile_pool(name="dram", bufs=1, space="DRAM") as dramp,
            tc.tile_pool(name="tload", bufs=3) as tloadp,
            tc.tile_pool(name="stream", bufs=8) as streamp,
            tc.tile_pool(name="gat", bufs=8) as gatp,
            tc.tile_pool(name="sall", bufs=5) as sallp,
            tc.tile_pool(name="sje", bufs=4) as sjep,
            tc.tile_pool(name="work", bufs=6) as workp,
            tc.tile_pool(name="psT", bufs=2, space="PSUM") as psT,
            tc.tile_pool(name="psM", bufs=3, space="PSUM") as psM,
            tc.tile_pool(name="psA", bufs=3, space="PSUM") as psA,
        ):
            # ---------------- constants
            iota_pm = constp.tile([128, 128], dt.int16, tag="iota_pm")
            nc.gpsimd.iota(iota_pm[:], pattern=[[1, 128]], base=0, channel_multiplier=-1)
            iota_pmf = constp.tile([128, 128], dt.float32, tag="iota_pmf")
            nc.vector.tensor_copy(iota_pmf[:], iota_pm[:])
            ident_b = constp.tile([128, 128], dt.bfloat16, tag="ident_b")
            nc.vector.tensor_scalar(out=ident_b[:], in0=iota_pmf[:], scalar1=0.0,
                                    scalar2=None, op0=OP.is_equal)

            wcat_b = constp.tile([128, 128], dt.bfloat16, tag="wcat_b")
            nc.sync.dma_start(wcat_b[:], w_cat[:])
            webd_b = constp.tile([88, GS * DOUT], dt.bfloat16, tag="webd_b")
            nc.sync.dma_start(webd_b[:], w_ebd[:])
            attb_b = constp.tile([128, DOUT], dt.bfloat16, tag="attb_b")
            nc.sync.dma_start(attb_b[:], att_bc[:])

            xr_own = ownp.tile([128, NCHUNK, DOUT], dt.bfloat16, tag="xr_own")
            agg_all = ownp.tile([128, NCHUNK, RHS_W], dt.float32, tag="agg_all")


            C_tab = dramp.tile([NTAB, 128], dt.bfloat16)

            # ---------------- phase 1a: node table C[n] = [xl(n) | xr(n)]
            # xT column (s*2048 + j*128 + p) holds node with table row
            # (s*128 + p)*16 + j, so each slab writes 2048 contiguous elems
            # per partition (fat DMA descriptors).
            for s in range(NSLAB):
                xt_b = tloadp.tile([128, 2048], dt.bfloat16, tag="xt_b")
                nc.sync.dma_start(xt_b[:], xT[:, s * 2048:(s + 1) * 2048])
                cs = tloadp.tile([128, 2048], dt.bfloat16, tag="cs")
                for q in range(4):
                    ps = psM.tile([128, 512], dt.float32, tag="psM")
                    for j in range(4):
                        nc.tensor.matmul(
                            ps[:, j * 128:(j + 1) * 128],
                            lhsT=xt_b[:, (q * 4 + j) * 128:(q * 4 + j + 1) * 128],
                            rhs=wcat_b[:], start=True, stop=True)
                    nc.vector.tensor_copy(cs[:, q * 512:(q + 1) * 512], ps[:])
                # den column baked into the table: row = [xl | 1 | junk]
                nc.vector.memset(
                    cs[:, :].rearrange("p (n f) -> p n f", f=128)[:, :, DOUT:DOUT + 1],
                    1.0)
                nc.sync.dma_start(C_tab[s * 2048:(s + 1) * 2048, :], cs[:])

            # ---------------- phase 1b: xr for own nodes
            for cb in range(0, NCHUNK, 8):
                nch = min(8, NCHUNK - cb)
                xo_b = tloadp.tile([128, 1024], dt.bfloat16, tag="xo_b")
                nc.sync.dma_start(xo_b[:, 0:nch * 128],
                                  x_ownT[:, cb * 128:(cb + nch) * 128])
                ps = psM.tile([128, 512], dt.float32, tag="psM")
                for j in range(nch):
                    nc.tensor.matmul(ps[:, j * 64:(j + 1) * 64],
                                     lhsT=xo_b[:, j * 128:(j + 1) * 128],
                                     rhs=wcat_b[:, 64:128], start=True, stop=True)
                nc.scalar.copy(xr_own[:, cb:cb + nch, :], ps[:, 0:nch * 64])

            # ---------------- phase 2: per-chunk edge pipeline
            for ch in range(NCHUNK):
                T = int(t_ch[ch])
                tb = int(tile_base[ch])
                gb = int(group_base[ch])
                ngr = _cdiv(T, GS)

                idxl = streamp.tile([128, tmax * 8], dt.int16, tag="idxl")
                nc.sync.dma_start(idxl[:, 0:T * 8], idx_xl[:, tb * 8:(tb + T) * 8])
                a8 = streamp.tile([128, ngmax * 128], dt.bfloat16, tag="a8")
                nc.sync.dma_start(a8[0:88, 0:ngr * 128],
                                  attr8_t[:, gb * 128:(gb + ngr) * 128])
                sa = sallp.tile([128, tmax, 128], dt.bfloat16, tag="sa")
                nc.sync.dma_start(sa[:, 0:T, :].rearrange("p t d -> p (t d)"),
                                  sall_d[:, tb * 128:(tb + T) * 128])

                g = gatp.tile([128, tmax, 128], dt.bfloat16, tag="g")
                if ch < 8:  # first uses of the rotating slots: clear stale bits
                    nc.vector.memset(g[:, :, :], 0.0)
                for b in range(BANKS):
                    ncb = int(ntiles_cb[ch, b])
                    if ncb == 0:
                        continue
                    mc = int(maxcnt[ch, b])
                    bofs = int(cto[ch, b])
                    nc.gpsimd.dma_gather(
                        out_ap=g[:, bofs:bofs + ncb, :],
                        in_ap=C_tab[b * BANKROWS:(b + 1) * BANKROWS, :],
                        idxs_ap=idxl[:, bofs * 8:bofs * 8 + _cdiv(mc, 16)],
                        num_idxs=mc, num_idxs_reg=mc, elem_size=128,
                        queue_num=next_q())

                agg = psA.tile([128, RHS_W], dt.float32, tag="psA")
                for gi in range(ngr):
                    t0 = gi * GS
                    ngt = min(GS, T - t0)

                    pst = psT.tile([128, GS * 128], dt.bfloat16, tag="psT")
                    for k in range(ngt):
                        nc.tensor.transpose(pst[:, k * 128:(k + 1) * 128],
                                            in_=sa[:, t0 + k, :], identity=ident_b[:])
                    sje = sjep.tile([128, GS * 128], dt.bfloat16, tag="sje")
                    nc.scalar.copy(sje[:, 0:ngt * 128], pst[:, 0:ngt * 128])

                    # psm = attr @ We (block-diag) + xr[dst] + xl[src]
                    pm = psM.tile([128, 512], dt.float32, tag="psM")
                    nc.tensor.matmul(pm[:, 0:ngt * 64],
                                     lhsT=a8[0:88, gi * 128:(gi + 1) * 128],
                                     rhs=webd_b[:, 0:ngt * 64],
                                     start=True, stop=False)
                    for k in range(ngt):
                        nc.tensor.matmul(pm[:, k * 64:(k + 1) * 64],
                                         lhsT=sje[:, k * 128:(k + 1) * 128],
                                         rhs=xr_own[:, ch, :],
                                         start=False, stop=False)
                    for k in range(ngt):
                        nc.tensor.matmul(pm[:, k * 64:(k + 1) * 64],
                                         lhsT=ident_b[:],
                                         rhs=g[:, t0 + k, 0:DOUT],
                                         start=False, stop=True)

                    mrelu = workp.tile([128, GS, DOUT], dt.bfloat16, tag="mrelu")
                    nc.scalar.activation(
                        mrelu[:, 0:ngt, :].rearrange("p t d -> p (t d)"),
                        pm[:, 0:ngt * 64], AF.Prelu, alpha=NEG_SLOPE)
                    lt = workp.tile([128, GS, DOUT], dt.bfloat16, tag="lt")
                    nc.vector.tensor_tensor(out=lt[:, 0:ngt, :],
                                            in0=mrelu[:, 0:ngt, :],
                                            in1=_bc3(attb_b[:, :], ngt), op=OP.mult)
                    logits = workp.tile([128, GS], dt.float32, tag="logits")
                    nc.vector.tensor_reduce(out=logits[:, 0:ngt], in_=lt[:, 0:ngt, :],
                                            axis=mybir.AxisListType.X, op=OP.add)
                    ex = workp.tile([128, GS], dt.bfloat16, tag="ex")
                    nc.scalar.activation(ex[:, 0:ngt], logits[:, 0:ngt], AF.Exp)

                    # rhs = ex * [xl | 1] (the 1 is baked into table column 64)
                    rhs = workp.tile([128, GS, RHS_W], dt.bfloat16, tag="rhs")
                    nc.vector.tensor_tensor(out=rhs[:, 0:ngt, :],
                                            in0=g[:, t0:t0 + ngt, 0:RHS_W],
                                            in1=_in3(ex[:, 0:ngt], RHS_W), op=OP.mult)

                    for k in range(ngt):
                        nc.tensor.matmul(agg[:], lhsT=sa[:, t0 + k, :],
                                         rhs=rhs[:, k, :],
                                         start=(t0 + k == 0), stop=(t0 + k == T - 1))

                nc.vector.tensor_copy(agg_all[:, ch, :], agg[:])

            # ---------------- epilogue: out = num / den
            rden = ownp.tile([128, NCHUNK], dt.float32, tag="rden")
            nc.vector.reciprocal(rden[:], agg_all[:, :, DOUT])
            o1 = ownp.tile([128, NCHUNK, DOUT], dt.float32, tag="o1")
            nc.vector.tensor_tensor(out=o1[:], in0=agg_all[:, :, 0:DOUT],
                                    in1=_in3(rden[:, :], DOUT), op=OP.mult)
            nc.sync.dma_start(out_d[:, :],
                              o1[:, :, :].rearrange("p t d -> p (t d)"))

    nc.compile()
    return nc


# --------------------------------------------------------------------------
# entry point
# --------------------------------------------------------------------------

def kernel(x, edge_index, edge_attr, W_l, W_r, W_e, att):
    global last_exec_time_ns, last_result
    x = np.asarray(x, np.float32)
    edge_attr = np.asarray(edge_attr, np.float32)
    W_l = np.asarray(W_l, np.float32)
    W_r = np.asarray(W_r, np.float32)
    W_e = np.asarray(W_e, np.float32)
    att = np.asarray(att, np.float32)

    plan = _plan(edge_index)

    # self-loop attrs = mean of incoming edge attrs per destination node
    dst = np.asarray(edge_index[1]).astype(np.int64)
    deg = plan["deg"].astype(np.float32)
    loop_attr = np.empty((N, DE), np.float32)
    for a in range(DE):
        loop_attr[:, a] = np.bincount(dst, weights=edge_attr[:, a], minlength=N)
    loop_attr /= np.maximum(deg, 1.0)[:, None]
    attr_f = np.concatenate([edge_attr, loop_attr], axis=0)

    per_core = _host_arrays(plan, edge_attr, attr_f)

    key = plan["maxcnt"].tobytes() + plan["t_ch"].tobytes()
    if key not in _CACHE:
        _CACHE[key] = _build_device(plan)
    nc = _CACHE[key]

    # xT column (s*2048 + j*128 + p) holds node (s*128 + p)*16 + j
    cidx = np.arange(NTAB)
    node_for_col = ((cidx // 2048) * 128 + (cidx % 128)) * 16 + (cidx % 2048) // 128
    xpad = np.zeros((NTAB, DIN), np.float32)
    xpad[:N] = x
    xT = xpad[node_for_col].T.astype(BF16)

    w_cat = np.concatenate([W_l, W_r], axis=1).astype(BF16)
    w_ebd = np.zeros((88, GS * DOUT), np.float32)
    for k in range(GS):
        w_ebd[11 * k:11 * (k + 1), 64 * k:64 * (k + 1)] = W_e
    w_ebd = w_ebd.astype(BF16)
    att_bc = np.tile(att[None, :], (128, 1)).astype(BF16)

    node_of = plan["node_of"]
    in_maps = []
    for c in range(NC):
        x_ownT = np.zeros((128, NPAD), BF16)
        x_ownT[:, :NOWN] = x[node_of[c]].T.astype(BF16)
        pc = per_core[c]
        in_maps.append({
            "xT": xT, "x_ownT": x_ownT, "w_cat": w_cat, "w_ebd": w_ebd,
            "att_bc": att_bc, "idx_xl": pc["idx_xl"],
            "sall": pc["sall"], "attr8": pc["attr8"],
        })

    try:
        res = run_bass_kernel_spmd(nc, in_maps, core_ids=list(range(NC)), trace=True)
        last_exec_time_ns = res.exec_time_ns
    except Exception:
        res = run_bass_kernel_spmd(nc, in_maps, core_ids=list(range(NC)), trace=False)
        last_exec_time_ns = None
    last_result = res

    out = np.zeros((N, DOUT), np.float32)
    for c in range(NC):
        r = res.results[c]["out"].reshape(128, NCHUNK, DOUT)
        r = r.transpose(1, 0, 2).reshape(NPAD, DOUT)
        out[node_of[c]] = r[:NOWN]
    return out


# revision 25
# speedup vs baseline: 1.1845x; 1.0275x over previous
"""GATv2Conv (heads=1, edge_dim=11, add_self_loops fill='mean') on 8 Trainium2 cores.

Sharding: nodes dealt to cores by degree rank (core = rank % 8), so per-cell
edge counts are nearly identical across cores and the shared-NEFF padding is
minimal. Edges grouped per (dst-chunk, src-bank) cell; per-edge xl[src] rows
come from one dma_gather per cell out of an on-device bf16 table C[n]=[xl|xr].
Self-loops are folded in as ordinary edges (attr = host-computed mean).

Per chunk, tiles are processed in groups of 8: the one-hot S arrives pre-built
from the host (bf16 DMA), is transposed on the PE (batched PSUM->SBUF copy),
messages are assembled in PSUM via bf16 matmuls only (block-diagonal attr@We +
S^T@xr + identity@xl), Prelu/logits/exp run on ACT+DVE, the softmax weight is
applied once to the 65-wide rhs [xl | 1] (the 1 baked into table column 64),
and segment sums (numerator and denominator together) are S^T @ rhs matmuls.

All fp32 has been removed from the DMA and matmul paths (bf16 end to end,
fp32 only in PSUM accumulation and the final divide). Host work is layout
only: sharding, grouping, index packing, casts, and the self-loop attr mean.
"""

import sys

sys.path.insert(0, "/opt/trn_rl_repo")

import numpy as np
import ml_dtypes

import concourse.bass as bass
import concourse.bacc as bacc
import concourse.tile as tile
import concourse.mybir as mybir
from concourse.bass_utils import run_bass_kernel_spmd

BF16 = ml_dtypes.bfloat16
AF = mybir.ActivationFunctionType
OP = mybir.AluOpType

N, E, DIN, DOUT, DE = 1000_00, 1000_000, 128, 64, 11
NEG_SLOPE = 0.2
NC = 8
NOWN = N // NC                    # 12500
NCHUNK = 98                       # ceil(12500/128)
NPAD = NCHUNK * 128               # 12544
BANKS = 4
BANKROWS = 25024                  # 4*25024 = 100096 >= N; < 2^15 for int16 idx
NSLAB = 49                        # table build slabs of 2048 rows
NTAB = NSLAB * 2048               # 100352 >= 100096
GS = 8                            # tiles per processing group
RHS_W = 65                        # [ex | ex*xl]

last_exec_time_ns = None
last_result = None
_CACHE = {}


def _cdiv(a, b):
    return -(-a // b)


def _bc3(ap2, mid):
    """[P, F] AP -> [P, mid, F] AP broadcast along a new middle dim."""
    return bass.AP(ap2.tensor, ap2.offset, [ap2.ap[0], [0, mid], ap2.ap[1]])


def _in3(ap2, inner):
    """[P, T] AP -> [P, T, inner] AP broadcast along a new inner dim."""
    return bass.AP(ap2.tensor, ap2.offset, [ap2.ap[0], ap2.ap[1], [0, inner]])


# --------------------------------------------------------------------------
# host-side layout (index manipulation only)
# --------------------------------------------------------------------------

def _plan(edge_index):
    src = np.asarray(edge_index[0]).astype(np.int64)
    dst = np.asarray(edge_index[1]).astype(np.int64)
    deg = np.bincount(dst, minlength=N).astype(np.int64)

    # deal nodes to cores by degree rank: nearly equal per-cell counts
    order = np.argsort(deg, kind="stable")        # node ids, ascending degree
    core_of = np.empty(N, np.int64)
    slot_of = np.empty(N, np.int64)
    core_of[order] = np.arange(N) % NC
    slot_of[order] = np.arange(N) // NC           # 0..12499
    node_of = np.empty((NC, NOWN), np.int64)      # (core, slot) -> node id
    node_of[np.arange(N) % NC, np.arange(N) // NC] = order

    # append self loops as ordinary edges
    loop = np.arange(N, dtype=np.int64)
    src_f = np.concatenate([src, loop])           # [E+N]
    dst_f = np.concatenate([dst, loop])

    ecore = core_of[dst_f]
    eslot = slot_of[dst_f]
    chunk = eslot >> 7
    ldst = eslot & 127
    bank = src_f // BANKROWS
    cell = (chunk * BANKS + bank)

    counts = np.zeros((NC, NCHUNK * BANKS), np.int64)
    for c in range(NC):
        counts[c] = np.bincount(cell[ecore == c], minlength=NCHUNK * BANKS)
    maxcnt = counts.max(axis=0).reshape(NCHUNK, BANKS)
    ntiles_cb = (maxcnt + 127) // 128
    t_ch = ntiles_cb.sum(axis=1)                  # tiles per chunk
    tile_base = np.zeros(NCHUNK + 1, np.int64)
    tile_base[1:] = np.cumsum(t_ch)
    ng_ch = (t_ch + GS - 1) // GS                 # groups per chunk
    group_base = np.zeros(NCHUNK + 1, np.int64)
    group_base[1:] = np.cumsum(ng_ch)
    cell_tile_ofs = np.cumsum(ntiles_cb, axis=1) - ntiles_cb

    # global ordering of edges: (core, chunk, bank), stable
    key = (ecore * (NCHUNK * BANKS) + cell)
    eorder = np.argsort(key, kind="stable")
    key_s = key[eorder]
    starts = np.zeros(NC * NCHUNK * BANKS + 1, np.int64)
    starts[1:] = np.cumsum(np.bincount(key_s, minlength=NC * NCHUNK * BANKS))
    rank = np.arange(src_f.shape[0], dtype=np.int64) - starts[key_s]

    return dict(src_f=src_f, dst_f=dst_f, deg=deg, node_of=node_of,
                ecore=ecore, chunk=chunk, ldst=ldst, bank=bank,
                eorder=eorder, rank=rank,
                maxcnt=maxcnt, ntiles_cb=ntiles_cb, t_ch=t_ch,
                tile_base=tile_base, ng_ch=ng_ch, group_base=group_base,
                cell_tile_ofs=cell_tile_ofs,
                tot_tiles=int(t_ch.sum()), tot_groups=int(ng_ch.sum()),
                tmax=int(t_ch.max()), ngmax=int(ng_ch.max()))


def _host_arrays(plan, edge_attr, attr_f=None):
    """Per-core idx/ldst/attr streams. attr_f = full per-edge attrs [E+N, DE]."""
    tot_tiles, tot_groups = plan["tot_tiles"], plan["tot_groups"]
    idxcols = tot_tiles * 8
    eorder = plan["eorder"]
    tile_base, cto = plan["tile_base"], plan["cell_tile_ofs"]
    group_base = plan["group_base"]

    ecore_o = plan["ecore"][eorder]
    chunk_o = plan["chunk"][eorder]
    ldst_o = plan["ldst"][eorder]
    bank_o = plan["bank"][eorder]
    src_o = plan["src_f"][eorder]
    rank_o = plan["rank"]                 # already in sorted (eorder) order
    attr_o = attr_f[eorder]

    cell_t0 = tile_base[chunk_o] + cto[chunk_o, bank_o]
    tile_abs = cell_t0 + rank_o // 128
    part = rank_o % 128
    ofs16 = cell_t0 * 8 + rank_o // 16
    r16 = rank_o % 16
    tloc = tile_abs - tile_base[chunk_o]
    gcol = (group_base[chunk_o] + tloc // GS) * 128 + part
    krow = 11 * (tloc % GS)

    per_core = []
    for c in range(NC):
        m = ecore_o == c

        i16 = np.zeros((16, idxcols), np.int16)
        i16[r16[m], ofs16[m]] = (src_o[m] - bank_o[m] * BANKROWS).astype(np.int16)
        idx_xl = np.tile(i16, (8, 1))

        sall = np.zeros((128, tot_tiles * 128), BF16)
        sall[part[m], tile_abs[m] * 128 + ldst_o[m]] = 1.0

        attr8 = np.zeros((88, tot_groups * 128), BF16)
        attr8[(krow[m][:, None] + np.arange(DE)[None, :]),
              gcol[m][:, None]] = attr_o[m].astype(BF16)

        per_core.append(dict(idx_xl=idx_xl, sall=sall, attr8=attr8))
    return per_core


# --------------------------------------------------------------------------
# device program (one SPMD NEFF for 8 cores; layout baked from `plan`)
# --------------------------------------------------------------------------

def _build_device(plan):
    dt = mybir.dt
    maxcnt = plan["maxcnt"]
    ntiles_cb = plan["ntiles_cb"]
    t_ch = plan["t_ch"]
    tile_base = plan["tile_base"]
    group_base = plan["group_base"]
    cto = plan["cell_tile_ofs"]
    tot_tiles = plan["tot_tiles"]
    tot_groups = plan["tot_groups"]
    tmax, ngmax = plan["tmax"], plan["ngmax"]
    assert tmax <= 32, f"tmax={tmax}"

    nc = bacc.Bacc("TRN2", target_bir_lowering=False, debug=False,
                   num_devices=NC, num_swdge_queues=4)

    xT = nc.dram_tensor("xT", [128, NTAB], dt.bfloat16, kind="ExternalInput")
    x_ownT = nc.dram_tensor("x_ownT", [128, NPAD], dt.bfloat16, kind="ExternalInput")
    w_cat = nc.dram_tensor("w_cat", [128, 128], dt.bfloat16, kind="ExternalInput")
    w_ebd = nc.dram_tensor("w_ebd", [88, GS * DOUT], dt.bfloat16, kind="ExternalInput")
    att_bc = nc.dram_tensor("att_bc", [128, DOUT], dt.bfloat16, kind="ExternalInput")
    idx_xl = nc.dram_tensor("idx_xl", [128, tot_tiles * 8], dt.int16, kind="ExternalInput")
    sall_d = nc.dram_tensor("sall", [128, tot_tiles * 128], dt.bfloat16, kind="ExternalInput")
    attr8_t = nc.dram_tensor("attr8", [88, tot_groups * 128], dt.bfloat16, kind="ExternalInput")
    out_d = nc.dram_tensor("out", [128, NCHUNK * DOUT], dt.float32, kind="ExternalOutput")

    qn = [0]

    def next_q():
        q = qn[0] & 3
        qn[0] += 1
        return q

    with tile.TileContext(nc) as tc:
        with (
            tc.tile_pool(name="const", bufs=1) as constp,
            tc.tile_pool(name="own", bufs=1) as ownp,
            tc.tile_pool(name="dram", bufs=1, space="DRAM") as dramp,
            tc.tile_pool(name="tload", bufs=3) as tloadp,
            tc.tile_pool(name="stream", bufs=8) as streamp,
            tc.tile_pool(name="gat", bufs=8) as gatp,
            tc.tile_pool(name="sall", bufs=5) as sallp,
            tc.tile_pool(name="sje", bufs=4) as sjep,
            tc.tile_pool(name="work", bufs=6) as workp,
            tc.tile_pool(name="psT", bufs=2, space="PSUM") as psT,
            tc.tile_pool(name="psM", bufs=3, space="PSUM") as psM,
            tc.tile_pool(name="psA", bufs=3, space="PSUM") as psA,
        ):
            # ---------------- constants
            iota_pm = constp.tile([128, 128], dt.int16, tag="iota_pm")
            nc.gpsimd.iota(iota_pm[:], pattern=[[1, 128]], base=0, channel_multiplier=-1)
            iota_pmf = constp.tile([128, 128], dt.float32, tag="iota_pmf")
            nc.vector.tensor_copy(iota_pmf[:], iota_pm[:])
            ident_b = constp.tile([128, 128], dt.bfloat16, tag="ident_b")
            nc.vector.tensor_scalar(out=ident_b[:], in0=iota_pmf[:], scalar1=0.0,
                                    scalar2=None, op0=OP.is_equal)

            wcat_b = constp.tile([128, 128], dt.bfloat16, tag="wcat_b")
            nc.sync.dma_start(wcat_b[:], w_cat[:])
            webd_b = constp.tile([88, GS * DOUT], dt.bfloat16, tag="webd_b")
            nc.sync.dma_start(webd_b[:], w_ebd[:])
            attb_b = constp.tile([128, DOUT], dt.bfloat16, tag="attb_b")
            nc.sync.dma_start(attb_b[:], att_bc[:])

            xr_own = ownp.tile([128, NCHUNK, DOUT], dt.bfloat16, tag="xr_own")
            agg_all = ownp.tile([128, NCHUNK, RHS_W], dt.float32, tag="agg_all")


            C_tab = dramp.tile([NTAB, 128], dt.bfloat16)

            # ---------------- phase 1a: node table C[n] = [xl(n) | xr(n)]
            # xT column (s*2048 + j*128 + p) holds node with table row
            # (s*128 + p)*16 + j, so each slab writes 2048 contiguous elems
            # per partition (fat DMA descriptors).
            for s in range(NSLAB):
                xt_b = tloadp.tile([128, 2048], dt.bfloat16, tag="xt_b")
                nc.sync.dma_start(xt_b[:], xT[:, s * 2048:(s + 1) * 2048])
                cs = tloadp.tile([128, 2048], dt.bfloat16, tag="cs")
                for q in range(4):
                    ps = psM.tile([128, 512], dt.float32, tag="psM")
                    for j in range(4):
                        nc.tensor.matmul(
                            ps[:, j * 128:(j + 1) * 128],
                            lhsT=xt_b[:, (q * 4 + j) * 128:(q * 4 + j + 1) * 128],
                            rhs=wcat_b[:], start=True, stop=True)
                    nc.vector.tensor_copy(cs[:, q * 512:(q + 1) * 512], ps[:])
                # den column baked into the table: row = [xl | 1 | junk]
                nc.vector.memset(
                    cs[:, :].rearrange("p (n f) -> p n f", f=128)[:, :, DOUT:DOUT + 1],
                    1.0)
                nc.sync.dma_start(C_tab[s * 2048:(s + 1) * 2048, :], cs[:])

            # ---------------- phase 1b: xr for own nodes
            for cb in range(0, NCHUNK, 8):
                nch = min(8, NCHUNK - cb)
                xo_b = tloadp.tile([128, 1024], dt.bfloat16, tag="xo_b")
                nc.sync.dma_start(xo_b[:, 0:nch * 128],
                                  x_ownT[:, cb * 128:(cb + nch) * 128])
                ps = psM.tile([128, 512], dt.float32, tag="psM")
                for j in range(nch):
                    nc.tensor.matmul(ps[:, j * 64:(j + 1) * 64],
                                     lhsT=xo_b[:, j * 128:(j + 1) * 128],
                                     rhs=wcat_b[:, 64:128], start=True, stop=True)
                nc.scalar.copy(xr_own[:, cb:cb + nch, :], ps[:, 0:nch * 64])

            # ---------------- phase 2: per-chunk edge pipeline
            for ch in range(NCHUNK):
                T = int(t_ch[ch])
                tb = int(tile_base[ch])
                gb = int(group_base[ch])
                ngr = _cdiv(T, GS)

                idxl = streamp.tile([128, tmax * 8], dt.int16, tag="idxl")
                nc.sync.dma_start(idxl[:, 0:T * 8], idx_xl[:, tb * 8:(tb + T) * 8])
                a8 = streamp.tile([128, ngmax * 128], dt.bfloat16, tag="a8")
                nc.sync.dma_start(a8[0:88, 0:ngr * 128],
                                  attr8_t[:, gb * 128:(gb + ngr) * 128])
                sa = sallp.tile([128, tmax, 128], dt.bfloat16, tag="sa")
                nc.sync.dma_start(sa[:, 0:T, :].rearrange("p t d -> p (t d)"),
                                  sall_d[:, tb * 128:(tb + T) * 128])

                g = gatp.tile([128, tmax, 128], dt.bfloat16, tag="g")
                if ch < 8:  # first uses of the rotating slots: clear stale bits
                    nc.vector.memset(g[:, :, :], 0.0)
                for b in range(BANKS):
                    ncb = int(ntiles_cb[ch, b])
                    if ncb == 0:
                        continue
                    mc = int(maxcnt[ch, b])
                    bofs = int(cto[ch, b])
                    nc.gpsimd.dma_gather(
                        out_ap=g[:, bofs:bofs + ncb, :],
                        in_ap=C_tab[b * BANKROWS:(b + 1) * BANKROWS, :],
                        idxs_ap=idxl[:, bofs * 8:bofs * 8 + _cdiv(mc, 16)],
                        num_idxs=mc, num_idxs_reg=mc, elem_size=128,
                        queue_num=next_q())

                agg = psA.tile([128, RHS_W], dt.float32, tag="psA")
                for gi in range(ngr):
                    t0 = gi * GS
                    ngt = min(GS, T - t0)

                    pst = psT.tile([128, GS * 128], dt.bfloat16, tag="psT")
                    for k in range(ngt):
                        nc.tensor.transpose(pst[:, k * 128:(k + 1) * 128],
                                            in_=sa[:, t0 + k, :], identity=ident_b[:])
                    sje = sjep.tile([128, GS * 128], dt.bfloat16, tag="sje")
                    nc.scalar.copy(sje[:, 0:ngt * 128], pst[:, 0:ngt * 128])

                    # psm = attr @ We (block-diag) + xr[dst] + xl[src]
                    pm = psM.tile([128, 512], dt.float32, tag="psM")
                    nc.tensor.matmul(pm[:, 0:ngt * 64],
                                     lhsT=a8[0:88, gi * 128:(gi + 1) * 128],
                                     rhs=webd_b[:, 0:ngt * 64],
                                     start=True, stop=False)
                    for k in range(ngt):
                        nc.tensor.matmul(pm[:, k * 64:(k + 1) * 64],
                                         lhsT=sje[:, k * 128:(k + 1) * 128],
                                         rhs=xr_own[:, ch, :],
                                         start=False, stop=False)
                    for k in range(ngt):
                        nc.tensor.matmul(pm[:, k * 64:(k + 1) * 64],
                                         lhsT=ident_b[:],
                                         rhs=g[:, t0 + k, 0:DOUT],
                                         start=False, stop=True)

                    mrelu = workp.tile([128, GS, DOUT], dt.bfloat16, tag="mrelu")
                    nc.scalar.activation(
                        mrelu[:, 0:ngt, :].rearrange("p t d -> p (t d)"),
                        pm[:, 0:ngt * 64], AF.Prelu, alpha=NEG_SLOPE)
                    lt = workp.tile([128, GS, DOUT], dt.bfloat16, tag="lt")
                    nc.vector.tensor_tensor(out=lt[:, 0:ngt, :],
                                            in0=mrelu[:, 0:ngt, :],
                                            in1=_bc3(attb_b[:, :], ngt), op=OP.mult)
                    logits = workp.tile([128, GS], dt.float32, tag="logits")
                    nc.vector.tensor_reduce(out=logits[:, 0:ngt], in_=lt[:, 0:ngt, :],
                                            axis=mybir.AxisListType.X, op=OP.add)
                    ex = workp.tile([128, GS], dt.bfloat16, tag="ex")
                    nc.scalar.activation(ex[:, 0:ngt], logits[:, 0:ngt], AF.Exp)

                    # rhs = ex * [xl | 1] (the 1 is baked into table column 64)
                    rhs = workp.tile([128, GS, RHS_W], dt.bfloat16, tag="rhs")
                    nc.vector.tensor_tensor(out=rhs[:, 0:ngt, :],
                                            in0=g[:, t0:t0 + ngt, 0:RHS_W],
                                            in1=_in3(ex[:, 0:ngt], RHS_W), op=OP.mult)

                    for k in range(ngt):
                        nc.tensor.matmul(agg[:], lhsT=sa[:, t0 + k, :],
                                         rhs=rhs[:, k, :],
                                         start=(t0 + k == 0), stop=(t0 + k == T - 1))

                nc.vector.tensor_copy(agg_all[:, ch, :], agg[:])

            # ---------------- epilogue: out = num / den
            rden = ownp.tile([128, NCHUNK], dt.float32, tag="rden")
            nc.vector.reciprocal(rden[:], agg_all[:, :, DOUT])
            o1 = ownp.tile([128, NCHUNK, DOUT], dt.float32, tag="o1")
            nc.vector.tensor_tensor(out=o1[:], in0=agg_all[:, :, 0:DOUT],
                                    in1=_in3(rden[:, :], DOUT), op=OP.mult)
            nc.sync.dma_start(out_d[:, :],
                              o1[:, :, :].rearrange("p t d -> p (t d)"))

    nc.compile()
    return nc


# --------------------------------------------------------------------------
# entry point
# --------------------------------------------------------------------------

def kernel(x, edge_index, edge_attr, W_l, W_r, W_e, att):
    global last_exec_time_ns, last_result
    x = np.asarray(x, np.float32)
    edge_attr = np.asarray(edge_attr, np.float32)
    W_l = np.asarray(W_l, np.float32)
    W_r = np.asarray(W_r, np.float32)
    W_e = np.asarray(W_e, np.float32)
    att = np.asarray(att, np.float32)

    plan = _plan(edge_index)

    # self-loop attrs = mean of incoming edge attrs per destination node
    dst = np.asarray(edge_index[1]).astype(np.int64)
    deg = plan["deg"].astype(np.float32)
    loop_attr = np.empty((N, DE), np.float32)
    for a in range(DE):
        loop_attr[:, a] = np.bincount(dst, weights=edge_attr[:, a], minlength=N)
    loop_attr /= np.maximum(deg, 1.0)[:, None]
    attr_f = np.concatenate([edge_attr, loop_attr], axis=0)

    per_core = _host_arrays(plan, edge_attr, attr_f)

    key = plan["maxcnt"].tobytes() + plan["t_ch"].tobytes()
    if key not in _CACHE:
        _CACHE[key] = _build_device(plan)
    nc = _CACHE[key]

    # xT column (s*2048 + j*128 + p) holds node (s*128 + p)*16 + j
    cidx = np.arange(NTAB)
    node_for_col = ((cidx // 2048) * 128 + (cidx % 128)) * 16 + (cidx % 2048) // 128
    xpad = np.zeros((NTAB, DIN), np.float32)
    xpad[:N] = x
    xT = xpad[node_for_col].T.astype(BF16)

    w_cat = np.concatenate([W_l, W_r], axis=1).astype(BF16)
    w_ebd = np.zeros((88, GS * DOUT), np.float32)
    for k in range(GS):
        w_ebd[11 * k:11 * (k + 1), 64 * k:64 * (k + 1)] = W_e
    w_ebd = w_ebd.astype(BF16)
    att_bc = np.tile(att[None, :], (128, 1)).astype(BF16)

    node_of = plan["node_of"]
    in_maps = []
    for c in range(NC):
        x_ownT = np.zeros((128, NPAD), BF16)
        x_ownT[:, :NOWN] = x[node_of[c]].T.astype(BF16)
        pc = per_core[c]
        in_maps.append({
            "xT": xT, "x_ownT": x_ownT, "w_cat": w_cat, "w_ebd": w_ebd,
            "att_bc": att_bc, "idx_xl": pc["idx_xl"],
            "sall": pc["sall"], "attr8": pc["attr8"],
        })

    try:
        res = run_bass_kernel_spmd(nc, in_maps, core_ids=list(range(NC)), trace=True)
        last_exec_time_ns = res.exec_time_ns
    except Exception:
        res = run_bass_kernel_spmd(nc, in_maps, core_ids=list(range(NC)), trace=False)
        last_exec_time_ns = None
    last_result = res

    out = np.zeros((N, DOUT), np.float32)
    for c in range(NC):
        r = res.results[c]["out"].reshape(128, NCHUNK, DOUT)
        r = r.transpose(1, 0, 2).reshape(NPAD, DOUT)
        out[node_of[c]] = r[:NOWN]
    return out
